# revision 4
# baseline (speedup 1.0000x reference)
"""CondConv2d on 8 Trainium2 NeuronCores — data-parallel over batch N=8.

Per-core (one sample), all conv data in bf16:
  - x is loaded once per copy (lower partitions 0-63 = channels, upper 64-127 =
    the same channels row-shifted by one) in 5 uneven chunks; the last chunk is
    tiny (8 rows) so almost no reduction work remains after the load finishes.
  - The attention branch (three global-mean-pooled conv3ds) collapses to a
    linear function of 18 partial "basis" sums of x (5 chunk totals, edge
    rows/cols in chunk-aligned parts, corners, const).  All partials are
    chunk-gated DVE ops writing columns of one [64,18] matrix; the
    (channel x basis) coefficient contraction absorbs them in 4 fused DVE ops.
  - Softmax normalization is skipped: weights are mixed with raw exp(logits)
    (the static conv_w bank is pre-folded into each bank host-side since
    sum(att)=1), and the 1/sum(exp) scale is applied at PSUM eviction together
    with the conv bias.
  - The 3x3 conv runs as 43 PSUM tiles x 6 accumulating PE matmuls over a
    130-wide zero-padded layout; contraction 128 = 64 channels + 64 channels
    of the row-shifted copy, pairing taps (-1,w)+(0,w) per matmul.  Tiles are
    processed m-outer in groups of 6 (6 PSUM banks) so the conv starts as soon
    as bank 0 of the mixed weight is ready.
  - Dummy matmuls gated on chunk arrivals keep the PE p-state ramped through
    the load phase so the conv starts at full clock.
"""
import numpy as np

CONV_DT = "bf16"
N, C, H, W = 8, 64, 128, 128
K = 4
WP = W + 2                 # padded row width (130)
NELEM = WP * W + 262       # per-partition x buffer length (16902)
ROWS_PER_TILE = 3
GS = 6                     # conv tiles per PSUM group (m-outer inside)
NT = 43                    # ceil((H+1-1)/3) PSUM tiles

CHUNK_ROWS = [30, 30, 30, 30, 8]
CHUNK_OFF = [0, 30, 60, 90, 120, 128]          # row boundaries

MM_TAPS = [((-1, -1), (0, -1)), ((-1, 0), (0, 0)), ((-1, 1), (0, 1)),
           ((1, -1), None), ((1, 0), None), ((1, 1), None)]
MM_OFFS = [130 * L[0] + L[1] for L, _ in MM_TAPS]

NBASIS = 18


# ----------------------------------------------------------------------------
# host-side prep
# ----------------------------------------------------------------------------
def _make_cw2(net0_w, net0_b, net1_w, net1_b, net2_w, net2_b):
    """CW[c, b, k] over the 10 logical bases:
    0=total, 1=row0, 2=row127, 3=col0, 4=col127,
    5..8=corners (00,0W,H0,HW), 9=const 1."""
    cw = np.zeros((C, 10, K), np.float64)
    scale = 1.0 / (C * H * W)
    for w_net, pads in ((net0_w, (0, 0, 0)), (net1_w, (1, 1, 1)), (net2_w, (2, 1, 1))):
        Kk, _, kd, kh, kw = w_net.shape
        pd, ph, pw = pads
        for i in range(kd):
            clo, chi = max(0, i - pd), min(C - 1, C - 1 + i - pd)
            cmask = np.zeros(C)
            cmask[clo:chi + 1] = 1.0
            for j in range(kh):
                hlo, hhi = max(0, j - ph), min(H - 1, H - 1 + j - ph)
                dropA = 0 if hlo == 1 else (127 if hhi == H - 2 else None)
                for l in range(kw):
                    wlo, whi = max(0, l - pw), min(W - 1, W - 1 + l - pw)
                    dropB = 0 if wlo == 1 else (127 if whi == W - 2 else None)
                    v = np.zeros(10)
                    v[0] = 1.0
                    if dropA == 0: v[1] = -1.0
                    if dropA == 127: v[2] = -1.0
                    if dropB == 0: v[3] = -1.0
                    if dropB == 127: v[4] = -1.0
                    if dropA is not None and dropB is not None:
                        v[{(0, 0): 5, (0, 127): 6, (127, 0): 7, (127, 127): 8}[(dropA, dropB)]] = 1.0
                    for k in range(Kk):
                        cw[:, :, k] += w_net[k, 0, i, j, l] * scale * np.outer(cmask, v)
    btot = (net0_b + net1_b + net2_b).astype(np.float64)
    cw[:, 9, :] += btot[None, :] / C
    # expand to the 18 partial-sum columns actually produced on device:
    # 0..4 = chunk totals, 5 = row0, 6 = row127, 7..9 = col0 parts,
    # 10..12 = col127 parts, 13..16 = corners, 17 = const.
    exp_map = [0, 0, 0, 0, 0, 1, 2, 3, 3, 3, 4, 4, 4, 5, 6, 7, 8, 9]
    cwx = cw[:, exp_map, :]
    return np.ascontiguousarray(cwx.astype(np.float32))


def _make_bank(Wt):
    """Wt (co, ci, 3, 3) -> (128, 6, 64): [p=ci(lo)/64+ci(hi), mm, co]."""
    bank = np.zeros((128, 6, 64), np.float32)
    for m, (L, Hh) in enumerate(MM_TAPS):
        bank[:64, m, :] = Wt[:, :, 1 + L[0], 1 + L[1]].T
        if Hh is not None:
            bank[64:, m, :] = Wt[:, :, 1 + Hh[0], 1 + Hh[1]].T
    return bank


# ----------------------------------------------------------------------------
# device program
# ----------------------------------------------------------------------------
_NC_CACHE = {}


def _build_nc():
    import concourse.bacc as bacc
    import concourse.tile as tile
    from concourse import mybir

    f32 = mybir.dt.float32
    DT = mybir.dt.bfloat16
    Alu = mybir.AluOpType
    Ax = mybir.AxisListType
    Act = mybir.ActivationFunctionType

    nc = bacc.Bacc("TRN2", target_bir_lowering=False, debug=False,
                   enable_asserts=False, num_devices=N)
    xin = nc.dram_tensor("xin", [C, H * WP], DT, kind="ExternalInput")
    wbk = nc.dram_tensor("wbanks", [128, 6, K, 64], DT, kind="ExternalInput")
    cw2 = nc.dram_tensor("cw2", [C, NBASIS, K], f32, kind="ExternalInput")
    cb = nc.dram_tensor("convb", [C, 1], f32, kind="ExternalInput")
    outT = nc.dram_tensor("out", [C, H, W], f32, kind="ExternalOutput")

    with tile.TileContext(nc) as tc:
        with tc.tile_pool(name="singles", bufs=1) as S, \
             tc.tile_pool(name="stage", bufs=4) as STG, \
             tc.tile_pool(name="cpsum", bufs=GS, space="PSUM") as PS, \
             tc.tile_pool(name="spsum", bufs=1, space="PSUM") as PS1:

            XL = S.tile([128, NELEM], DT)
            wb_sb = S.tile([128, 6, K, 64], DT)
            cw2_sb = S.tile([C, NBASIS, K], f32)
            convb_sb = S.tile([C, 1], f32)
            zlhs = S.tile([128, 128], DT)
            onesall = S.tile([C, 128], f32)
            att_sb = S.tile([128, K], f32)
            M = S.tile([C, NBASIS], f32)
            G = S.tile([C, K], f32)
            gscr = S.tile([C, NBASIS], f32)
            mw = S.tile([128, 6, 64], f32)
            mwb = S.tile([128, 6, 64], DT)
            foldA = S.tile([C, 1952], DT)
            foldB = S.tile([C, 1952], DT)
            ssum = S.tile([128, 1], f32)
            sinv = S.tile([128, 1], f32)

            wpsum = PS1.tile([128, 512], f32)
            psum_b = PS1.tile([128, K], f32)

            # --- constants / border zeroing (all tiny) ---
            nc.vector.memset(zlhs, 0.0)
            nc.vector.memset(onesall, 1.0)
            nc.vector.memset(M[:, 17:18], 1.0)
            # borders: host pre-pads the row gaps; only head/tail need zeroing
            nc.vector.memset(XL[0:64, 0:132], 0.0)
            nc.vector.memset(XL[0:64, 132 + H * WP:NELEM], 0.0)
            nc.vector.memset(XL[64:128, 0:2], 0.0)
            nc.vector.memset(XL[64:128, 2 + H * WP:NELEM], 0.0)

            # --- PE pipeline warm-up (results discarded; zlhs is all-zero) ---
            for i in range(8):
                nc.tensor.matmul(wpsum[:, 0:128], zlhs, zlhs, start=True, stop=True)

            # --- x load: 5 uneven chunks; lower (parts 0-63) and row-shifted
            # upper copy (parts 64-127) in flight together.  All other input
            # DMAs are queued on the same ring AFTER the x chunks so x never
            # waits on them.
            for c in range(5):
                a = WP * CHUNK_OFF[c]
                ln = WP * CHUNK_ROWS[c]
                nc.sync.dma_start(out=XL[0:64, 132 + a: 132 + a + ln],
                                  in_=xin[:, a: a + ln])
                nc.sync.dma_start(out=XL[64:128, 2 + a: 2 + a + ln],
                                  in_=xin[:, a: a + ln])
            nc.sync.dma_start(out=cw2_sb, in_=cw2[:, :, :])
            nc.sync.dma_start(out=convb_sb, in_=cb[:, :])
            for m in range(6):
                nc.sync.dma_start(out=wb_sb[:, m, :, :], in_=wbk[:, m, :, :])

            # --- chunk-gated PE keep-warm dummies (read loaded regions) ---
            def dummies(base, span, n):
                step = max(1, (span - 390) // max(1, n - 1))
                for i in range(n):
                    off = 132 + base + i * step
                    nc.tensor.matmul(wpsum[:, 0:390], zlhs[0:64, :],
                                     XL[0:64, off:off + 390], start=True, stop=True)

            # --- attention basis partial sums, all chunk-gated on DVE ---
            # chunk c total -> M[:, c] via fused fold (2 elems/cycle)
            def fold(c, obuf):
                a = 132 + WP * CHUNK_OFF[c]
                ln = WP * CHUNK_ROWS[c]
                h = ln // 2
                nc.vector.scalar_tensor_tensor(
                    out=obuf[:, :h], in0=XL[0:64, a:a + h], scalar=1.0,
                    in1=XL[0:64, a + h:a + ln], op0=Alu.mult, op1=Alu.add,
                    accum_out=M[:, c:c + 1])

            def colpart(col, r0, r1, mcol):
                a = 132 + WP * r0 + col
                v = XL[0:64, a:a + WP * (r1 - r0)].rearrange(
                    "p (r w) -> p r w", w=WP)[:, :, 0:1]
                nc.vector.tensor_reduce(out=M[:, mcol:mcol + 1], in_=v,
                                        axis=Ax.XY, op=Alu.add)

            # after chunk 0: row0 sum, row-0 corners, total fold
            nc.vector.tensor_reduce(out=M[:, 5:6], in_=XL[0:64, 132:132 + W],
                                    axis=Ax.X, op=Alu.add)
            nc.vector.tensor_copy(
                out=M[:, 13:15].rearrange("p (a b) -> p a b", b=1),
                in_=XL[0:64, 132:132 + 254].rearrange("p (a b) -> p a b", b=127)[:, :, 0:1])
            fold(0, foldA)
            dummies(WP * 0, WP * 30, 8)
            # after chunk 1: fold + col parts rows [0,60)
            fold(1, foldB)
            colpart(0, 0, 60, 7)
            colpart(127, 0, 60, 10)
            dummies(WP * 30, WP * 30, 8)
            # after chunk 2: fold
            fold(2, foldA)
            dummies(WP * 60, WP * 30, 8)
            # after chunk 3: fold + col parts rows [60,120)
            fold(3, foldB)
            colpart(0, 60, 120, 8)
            colpart(127, 60, 120, 11)
            dummies(WP * 90, WP * 30, 12)
            # after chunk 4 (tiny): fold + row127 + col parts + corners
            fold(4, foldA)
            nc.vector.tensor_reduce(out=M[:, 6:7], in_=XL[0:64, 16642:16642 + W],
                                    axis=Ax.X, op=Alu.add)
            colpart(0, 120, 128, 9)
            colpart(127, 120, 128, 12)
            nc.vector.tensor_copy(
                out=M[:, 15:17].rearrange("p (a b) -> p a b", b=1),
                in_=XL[0:64, 16642:16642 + 254].rearrange("p (a b) -> p a b", b=127)[:, :, 0:1])

            # per-channel coefficient contraction: G[c,k] = sum_b M[c,b]*CW2[c,b,k]
            for k in range(K):
                nc.vector.scalar_tensor_tensor(
                    out=gscr, in0=M, scalar=1.0,
                    in1=cw2_sb[:, :, k], op0=Alu.mult, op1=Alu.mult,
                    accum_out=G[:, k:k + 1])

            # logits broadcast to all 128 partitions with one matmul
            nc.tensor.matmul(psum_b, onesall, G, start=True, stop=True)
            # keep PE hot while softmax + first bank mixing run on ACT/DVE
            dummies(WP * 90, WP * 30, 4)
            # unnormalized softmax: att = exp(logits); 1/sum applied at eviction
            nc.scalar.activation(out=att_sb, in_=psum_b, func=Act.Exp)

            # --- weight mixing: mwb[:,m,:] = sum_k exp_k * bank'_k[:,m,:] ---
            # (conv_w bank is pre-folded into each bank'_k host-side)
            def mixbank(m):
                nc.vector.tensor_scalar_mul(out=mw[:, m, :], in0=wb_sb[:, m, 0, :],
                                            scalar1=att_sb[:, 0:1])
                for k in range(1, K):
                    tgt = mwb if k == K - 1 else mw
                    nc.vector.scalar_tensor_tensor(
                        out=tgt[:, m, :], in0=wb_sb[:, m, k, :],
                        scalar=att_sb[:, k:k + 1], in1=mw[:, m, :],
                        op0=Alu.mult, op1=Alu.add)

            mixbank(0)
            nc.vector.tensor_reduce(out=ssum, in_=att_sb, axis=Ax.X, op=Alu.add)
            nc.vector.reciprocal(out=sinv, in_=ssum)
            for m in range(1, 6):
                mixbank(m)

            # --- main conv: m-outer in groups of GS tiles (one PSUM bank each);
            # eviction applies the 1/sum(exp) scale and conv bias together ---
            for g0 in range(0, NT, GS):
                tiles = list(range(g0, min(g0 + GS, NT)))
                nrows_t = {t: min(ROWS_PER_TILE, H + 1 - (1 + t * ROWS_PER_TILE))
                           for t in tiles}
                pts = {}
                for t in tiles:
                    pts[t] = PS.tile([64, WP * ROWS_PER_TILE], f32,
                                     tag="cps", name=f"cps{t}")[:, :WP * nrows_t[t]]
                for m in range(6):
                    for t in tiles:
                        r0 = 1 + t * ROWS_PER_TILE
                        F = WP * nrows_t[t]
                        rhs = XL[:, WP * r0 + MM_OFFS[m] + 1:
                                 WP * r0 + MM_OFFS[m] + 1 + F]
                        nc.tensor.matmul(pts[t], mwb[:, m, :], rhs,
                                         start=(m == 0), stop=(m == 5))
                    if m == 5:
                        for t in tiles:
                            r0 = 1 + t * ROWS_PER_TILE
                            F = WP * nrows_t[t]
                            st = STG.tile([64, WP * ROWS_PER_TILE], f32,
                                          tag="stg", name=f"stg{t}")
                            if t % 2 == 0:
                                nc.scalar.activation(out=st[:, :F], in_=pts[t],
                                                     func=Act.Identity,
                                                     bias=convb_sb[:, 0:1],
                                                     scale=sinv[0:64, 0:1])
                            else:
                                nc.vector.tensor_scalar(
                                    out=st[:, :F], in0=pts[t],
                                    scalar1=sinv[0:64, 0:1],
                                    scalar2=convb_sb[:, 0:1],
                                    op0=Alu.mult, op1=Alu.add)
                            src = st[:, :F].rearrange("p (r w) -> p r w",
                                                      w=WP)[:, :, 1:1 + W]
                            eng = nc.sync if t % 2 == 0 else nc.scalar
                            eng.dma_start(out=outT[:, r0 - 1:r0 - 1 + nrows_t[t], :],
                                          in_=src)

    nc.compile()
    return nc


def _get_nc():
    if "nc" not in _NC_CACHE:
        _NC_CACHE["nc"] = _build_nc()
    return _NC_CACHE["nc"]


def _prep_inputs(x, weight, conv_w, conv_b, net0_w, net0_b, net1_w, net1_b,
                 net2_w, net2_b):
    import ml_dtypes
    cw2 = _make_cw2(np.asarray(net0_w, np.float32), np.asarray(net0_b, np.float32),
                    np.asarray(net1_w, np.float32), np.asarray(net1_b, np.float32),
                    np.asarray(net2_w, np.float32), np.asarray(net2_b, np.float32))
    wf = np.asarray(weight, np.float32)
    cwf = np.asarray(conv_w, np.float32)
    # fold the static conv bank into every mixed bank (sum(att) == 1)
    banks = np.stack([_make_bank(wf[k] + cwf) for k in range(K)])  # (K,128,6,64)
    banks = np.ascontiguousarray(
        banks.transpose(1, 2, 0, 3)).astype(ml_dtypes.bfloat16)    # (128,6,K,64)
    convb = np.ascontiguousarray(np.asarray(conv_b, np.float32).reshape(C, 1))
    x = np.asarray(x, np.float32)
    xp = np.zeros((N, C, H, WP), np.float32)
    xp[:, :, :, :W] = x
    xs = xp.astype(ml_dtypes.bfloat16)
    in_maps = []
    for n in range(N):
        in_maps.append({
            "xin": np.ascontiguousarray(xs[n].reshape(C, H * WP)),
            "wbanks": banks,
            "cw2": cw2,
            "convb": convb,
        })
    return in_maps


def _run(inputs, trace=False, **kw):
    from concourse.bass_utils import run_bass_kernel_spmd
    nc = _get_nc()
    in_maps = _prep_inputs(**inputs)
    return run_bass_kernel_spmd(nc, in_maps, core_ids=list(range(N)), trace=trace, **kw)


def kernel(**inputs):
    res = _run(inputs)
    out = np.stack([res.results[n]["out"] for n in range(N)]).astype(np.float32)
    return out


# revision 10
# speedup vs baseline: 1.0592x; 1.0592x over previous
"""CondConv2d on 8 Trainium2 NeuronCores — data-parallel over batch N=8.

Per-core (one sample), all conv data in bf16:
  - x is loaded once per copy (lower partitions 0-63 = channels, upper 64-127 =
    the same channels row-shifted by one) in 5 uneven chunks; the last chunk is
    tiny (8 rows) so almost no reduction work remains after the load finishes.
  - The attention branch (three global-mean-pooled conv3ds) collapses to a
    linear function of 18 partial "basis" sums of x (5 chunk totals, edge
    rows/cols in chunk-aligned parts, corners, const).  All partials are
    chunk-gated DVE ops writing columns of one [64,18] matrix; the
    (channel x basis) coefficient contraction absorbs them in 4 fused DVE ops.
  - Softmax normalization is skipped: weights are mixed with raw exp(logits)
    (the static conv_w bank is pre-folded into each bank host-side since
    sum(att)=1), and the 1/sum(exp) scale is applied at PSUM eviction together
    with the conv bias.
  - The 3x3 conv runs as 43 PSUM tiles x 6 accumulating PE matmuls over a
    130-wide zero-padded layout; contraction 128 = 64 channels + 64 channels
    of the row-shifted copy, pairing taps (-1,w)+(0,w) per matmul.  Tiles are
    processed m-outer in groups of 6 (6 PSUM banks) so the conv starts as soon
    as bank 0 of the mixed weight is ready.
  - Dummy matmuls gated on chunk arrivals keep the PE p-state ramped through
    the load phase so the conv starts at full clock.
"""
import numpy as np

CONV_DT = "bf16"
N, C, H, W = 8, 64, 128, 128
K = 4
WP = W + 2                 # padded row width (130)
NELEM = WP * W + 262       # per-partition x buffer length (16902)
ROWS_PER_TILE = 3
GS = 6                     # conv tiles per PSUM group (m-outer inside)
NT = 43                    # ceil((H+1-1)/3) PSUM tiles

CHUNK_ROWS = [30, 30, 30, 30, 8]
CHUNK_OFF = [0, 30, 60, 90, 120, 128]          # row boundaries

MM_TAPS = [((-1, -1), (0, -1)), ((-1, 0), (0, 0)), ((-1, 1), (0, 1)),
           ((1, -1), None), ((1, 0), None), ((1, 1), None)]
MM_OFFS = [130 * L[0] + L[1] for L, _ in MM_TAPS]

NBASIS = 18


# ----------------------------------------------------------------------------
# host-side prep
# ----------------------------------------------------------------------------
def _make_cw2(net0_w, net0_b, net1_w, net1_b, net2_w, net2_b):
    """CW[c, b, k] over the 10 logical bases:
    0=total, 1=row0, 2=row127, 3=col0, 4=col127,
    5..8=corners (00,0W,H0,HW), 9=const 1."""
    cw = np.zeros((C, 10, K), np.float64)
    scale = 1.0 / (C * H * W)
    for w_net, pads in ((net0_w, (0, 0, 0)), (net1_w, (1, 1, 1)), (net2_w, (2, 1, 1))):
        Kk, _, kd, kh, kw = w_net.shape
        pd, ph, pw = pads
        for i in range(kd):
            clo, chi = max(0, i - pd), min(C - 1, C - 1 + i - pd)
            cmask = np.zeros(C)
            cmask[clo:chi + 1] = 1.0
            for j in range(kh):
                hlo, hhi = max(0, j - ph), min(H - 1, H - 1 + j - ph)
                dropA = 0 if hlo == 1 else (127 if hhi == H - 2 else None)
                for l in range(kw):
                    wlo, whi = max(0, l - pw), min(W - 1, W - 1 + l - pw)
                    dropB = 0 if wlo == 1 else (127 if whi == W - 2 else None)
                    v = np.zeros(10)
                    v[0] = 1.0
                    if dropA == 0: v[1] = -1.0
                    if dropA == 127: v[2] = -1.0
                    if dropB == 0: v[3] = -1.0
                    if dropB == 127: v[4] = -1.0
                    if dropA is not None and dropB is not None:
                        v[{(0, 0): 5, (0, 127): 6, (127, 0): 7, (127, 127): 8}[(dropA, dropB)]] = 1.0
                    for k in range(Kk):
                        cw[:, :, k] += w_net[k, 0, i, j, l] * scale * np.outer(cmask, v)
    btot = (net0_b + net1_b + net2_b).astype(np.float64)
    cw[:, 9, :] += btot[None, :] / C
    # expand to the 18 partial-sum columns actually produced on device:
    # 0..4 = chunk totals, 5 = row0, 6 = row127, 7..9 = col0 parts,
    # 10..12 = col127 parts, 13..16 = corners, 17 = const.
    exp_map = [0, 0, 0, 0, 0, 1, 2, 3, 3, 3, 4, 4, 4, 5, 6, 7, 8, 9]
    cwx = cw[:, exp_map, :]
    return np.ascontiguousarray(cwx.astype(np.float32))


def _make_bank(Wt):
    """Wt (co, ci, 3, 3) -> (128, 6, 64): [p=ci(lo)/64+ci(hi), mm, co]."""
    bank = np.zeros((128, 6, 64), np.float32)
    for m, (L, Hh) in enumerate(MM_TAPS):
        bank[:64, m, :] = Wt[:, :, 1 + L[0], 1 + L[1]].T
        if Hh is not None:
            bank[64:, m, :] = Wt[:, :, 1 + Hh[0], 1 + Hh[1]].T
    return bank


# ----------------------------------------------------------------------------
# device program
# ----------------------------------------------------------------------------
_NC_CACHE = {}


def _build_nc():
    import concourse.bacc as bacc
    import concourse.tile as tile
    from concourse import mybir

    f32 = mybir.dt.float32
    DT = mybir.dt.bfloat16
    Alu = mybir.AluOpType
    Ax = mybir.AxisListType
    Act = mybir.ActivationFunctionType

    nc = bacc.Bacc("TRN2", target_bir_lowering=False, debug=False,
                   enable_asserts=False, num_devices=N)
    xin = nc.dram_tensor("xin", [C, H * WP], DT, kind="ExternalInput")
    wbk = nc.dram_tensor("wbanks", [128, 6, K, 64], DT, kind="ExternalInput")
    cw2 = nc.dram_tensor("cw2", [C, NBASIS, K], f32, kind="ExternalInput")
    cb = nc.dram_tensor("convb", [C, 1], f32, kind="ExternalInput")
    outT = nc.dram_tensor("out", [C, H, W], f32, kind="ExternalOutput")

    with tile.TileContext(nc) as tc:
        with tc.tile_pool(name="singles", bufs=1) as S, \
             tc.tile_pool(name="stage", bufs=4) as STG, \
             tc.tile_pool(name="cpsum", bufs=GS, space="PSUM") as PS, \
             tc.tile_pool(name="spsum", bufs=1, space="PSUM") as PS1:

            XL = S.tile([128, NELEM], DT)
            wb_sb = S.tile([128, 6, K, 64], DT)
            cw2_sb = S.tile([C, NBASIS, K], f32)
            convb_sb = S.tile([C, 1], f32)
            zlhs = S.tile([128, 128], DT)
            onesall = S.tile([C, 128], f32)
            att_sb = S.tile([128, K], f32)
            M = S.tile([C, NBASIS], f32)
            G = S.tile([C, K], f32)
            gscr = S.tile([C, NBASIS], f32)
            mw = S.tile([128, 6, 64], f32)
            mwb = S.tile([128, 6, 64], DT)
            foldA = S.tile([C, 1952], DT)
            foldB = S.tile([C, 1952], DT)
            actscr = S.tile([C, 136], f32)
            attw = S.tile([128, 390], DT)
            ssum = S.tile([128, 1], f32)
            sinv = S.tile([128, 1], f32)

            wpsum = PS1.tile([128, 512], f32)
            psum_b = PS1.tile([128, K], f32)

            # --- constants / border zeroing (all tiny) ---
            nc.vector.memset(zlhs, 0.0)
            nc.vector.memset(onesall, 1.0)
            nc.vector.memset(M[:, 17:18], 1.0)
            # borders: host pre-pads the row gaps; only head/tail need zeroing
            nc.vector.memset(XL[0:64, 0:132], 0.0)
            nc.vector.memset(XL[0:64, 132 + H * WP:NELEM], 0.0)
            nc.vector.memset(XL[64:128, 0:2], 0.0)
            nc.vector.memset(XL[64:128, 2 + H * WP:NELEM], 0.0)

            # --- PE pipeline warm-up (results discarded; zlhs is all-zero) ---
            for i in range(8):
                nc.tensor.matmul(wpsum[:, 0:128], zlhs, zlhs, start=True, stop=True)

            # --- x load: 5 uneven chunks; lower (parts 0-63) and row-shifted
            # upper copy (parts 64-127) in flight together.  All other input
            # DMAs are queued on the same ring AFTER the x chunks so x never
            # waits on them.
            for c in range(5):
                a = WP * CHUNK_OFF[c]
                ln = WP * CHUNK_ROWS[c]
                nc.sync.dma_start(out=XL[0:64, 132 + a: 132 + a + ln],
                                  in_=xin[:, a: a + ln])
                nc.sync.dma_start(out=XL[64:128, 2 + a: 2 + a + ln],
                                  in_=xin[:, a: a + ln])
            nc.sync.dma_start(out=cw2_sb, in_=cw2[:, :, :])
            nc.sync.dma_start(out=convb_sb, in_=cb[:, :])
            for m in range(6):
                nc.sync.dma_start(out=wb_sb[:, m, :, :], in_=wbk[:, m, :, :])

            # --- chunk-gated PE keep-warm dummies (read loaded regions) ---
            def dummies(base, span, n):
                step = max(1, (span - 390) // max(1, n - 1))
                for i in range(n):
                    off = 132 + base + i * step
                    nc.tensor.matmul(wpsum[:, 0:390], zlhs[0:64, :],
                                     XL[0:64, off:off + 390], start=True, stop=True)

            # --- attention basis partial sums, all chunk-gated on DVE ---
            # chunk c total -> M[:, c] via fused fold (2 elems/cycle)
            def fold(c, obuf):
                a = 132 + WP * CHUNK_OFF[c]
                ln = WP * CHUNK_ROWS[c]
                h = ln // 2
                nc.vector.scalar_tensor_tensor(
                    out=obuf[:, :h], in0=XL[0:64, a:a + h], scalar=1.0,
                    in1=XL[0:64, a + h:a + ln], op0=Alu.mult, op1=Alu.add,
                    accum_out=M[:, c:c + 1])

            def colpart(col, r0, r1, mcol):
                a = 132 + WP * r0 + col
                v = XL[0:64, a:a + WP * (r1 - r0)].rearrange(
                    "p (r w) -> p r w", w=WP)[:, :, 0:1]
                nc.vector.tensor_reduce(out=M[:, mcol:mcol + 1], in_=v,
                                        axis=Ax.XY, op=Alu.add)

            # after chunk 0: row0 sum, row-0 corners, total fold
            nc.vector.tensor_reduce(out=M[:, 5:6], in_=XL[0:64, 132:132 + W],
                                    axis=Ax.X, op=Alu.add)
            nc.vector.tensor_copy(
                out=M[:, 13:15].rearrange("p (a b) -> p a b", b=1),
                in_=XL[0:64, 132:132 + 254].rearrange("p (a b) -> p a b", b=127)[:, :, 0:1])
            fold(0, foldA)
            dummies(WP * 0, WP * 30, 8)
            # after chunk 1: fold + col parts rows [0,60)
            fold(1, foldB)
            colpart(0, 0, 60, 7)
            colpart(127, 0, 60, 10)
            dummies(WP * 30, WP * 30, 8)
            # after chunk 2: fold
            fold(2, foldA)
            dummies(WP * 60, WP * 30, 8)
            # after chunk 3: fold + col parts rows [60,120)
            fold(3, foldB)
            colpart(0, 60, 120, 8)
            colpart(127, 60, 120, 11)
            dummies(WP * 90, WP * 30, 14)
            # after chunk 4 (tiny): DVE does the fold; the idle ACT engine
            # handles row127 / col parts / corners via activation-accumulate
            fold(4, foldA)
            nc.scalar.activation(out=actscr[:, 0:W], in_=XL[0:64, 16642:16642 + W],
                                 func=Act.Identity, accum_out=M[:, 6:7])
            for col, mcol in ((0, 9), (127, 12)):
                a = 132 + WP * 120 + col
                v = XL[0:64, a:a + WP * 8].rearrange("p (r w) -> p r w", w=WP)[:, :, 0:1]
                nc.scalar.activation(out=actscr[:, 128:136].rearrange(
                    "p (r w) -> p r w", w=1), in_=v,
                    func=Act.Identity, accum_out=M[:, mcol:mcol + 1])
            nc.scalar.copy(
                out=M[:, 15:17].rearrange("p (a b) -> p a b", b=1),
                in_=XL[0:64, 16642:16642 + 254].rearrange("p (a b) -> p a b", b=127)[:, :, 0:1])

            # per-channel coefficient contraction: G[c,k] = sum_b M[c,b]*CW2[c,b,k]
            for k in range(K):
                nc.vector.scalar_tensor_tensor(
                    out=gscr, in0=M, scalar=1.0,
                    in1=cw2_sb[:, :, k], op0=Alu.mult, op1=Alu.mult,
                    accum_out=G[:, k:k + 1])

            # logits broadcast to all 128 partitions with one matmul
            nc.tensor.matmul(psum_b, onesall, G, start=True, stop=True)
            # unnormalized softmax: att = exp(logits); 1/sum applied at eviction
            nc.scalar.activation(out=att_sb, in_=psum_b, func=Act.Exp)
            # PE keep-warm bridge across softmax+mixing: fillers whose deps
            # only clear late (att_sb after EXP, then a wide ACT-made scratch)
            att_bc = att_sb.bitcast(DT)
            for i in range(3):
                nc.tensor.matmul(wpsum[:, 0:2 * K], zlhs, att_bc, start=True, stop=True)
            nc.scalar.activation(out=attw, in_=XL[0:128, 132:132 + 390],
                                 func=Act.Identity, scale=att_sb[:, 0:1])
            for i in range(4):
                nc.tensor.matmul(wpsum[:, 0:390], zlhs, attw, start=True, stop=True)

            # --- weight mixing: mwb[:,m,:] = sum_k exp_k * bank'_k[:,m,:] ---
            # (conv_w bank is pre-folded into each bank'_k host-side)
            def mixbank(m):
                nc.vector.tensor_scalar_mul(out=mw[:, m, :], in0=wb_sb[:, m, 0, :],
                                            scalar1=att_sb[:, 0:1])
                for k in range(1, K):
                    tgt = mwb if k == K - 1 else mw
                    nc.vector.scalar_tensor_tensor(
                        out=tgt[:, m, :], in0=wb_sb[:, m, k, :],
                        scalar=att_sb[:, k:k + 1], in1=mw[:, m, :],
                        op0=Alu.mult, op1=Alu.add)

            mixbank(0)
            nc.vector.tensor_reduce(out=ssum, in_=att_sb, axis=Ax.X, op=Alu.add)
            nc.vector.reciprocal(out=sinv, in_=ssum)
            for m in range(1, 6):
                mixbank(m)

            # --- main conv: m-outer in groups of GS tiles (one PSUM bank each);
            # eviction applies the 1/sum(exp) scale and conv bias together ---
            def evict(t, pt):
                r0 = 1 + t * ROWS_PER_TILE
                nrows = min(ROWS_PER_TILE, H + 1 - r0)
                F = WP * nrows
                st = STG.tile([64, WP * ROWS_PER_TILE], f32,
                              tag="stg", name=f"stg{t}")
                if t % 2 == 0:
                    nc.scalar.activation(out=st[:, :F], in_=pt,
                                         func=Act.Identity,
                                         bias=convb_sb[:, 0:1],
                                         scale=sinv[0:64, 0:1])
                else:
                    nc.vector.tensor_scalar(
                        out=st[:, :F], in0=pt,
                        scalar1=sinv[0:64, 0:1],
                        scalar2=convb_sb[:, 0:1],
                        op0=Alu.mult, op1=Alu.add)
                src = st[:, :F].rearrange("p (r w) -> p r w", w=WP)[:, :, 1:1 + W]
                eng = nc.sync if t % 2 == 0 else nc.scalar
                eng.dma_start(out=outT[:, r0 - 1:r0 - 1 + nrows, :], in_=src)

            def conv_mm(t, pt, m):
                r0 = 1 + t * ROWS_PER_TILE
                F = WP * min(ROWS_PER_TILE, H + 1 - r0)
                rhs = XL[:, WP * r0 + MM_OFFS[m] + 1:
                         WP * r0 + MM_OFFS[m] + 1 + F]
                nc.tensor.matmul(pt, mwb[:, m, :], rhs,
                                 start=(m == 0), stop=(m == 5))

            def mktile(t):
                F = WP * min(ROWS_PER_TILE, H + 1 - (1 + t * ROWS_PER_TILE))
                return PS.tile([64, WP * ROWS_PER_TILE], f32,
                               tag="cps", name=f"cps{t}")[:, :F]

            # group 0 m-outer: the first matmuls only need mixed bank 0
            pts = {t: mktile(t) for t in range(GS)}
            for m in range(6):
                for t in range(GS):
                    conv_mm(t, pts[t], m)
            for t in range(GS):
                evict(t, pts[t])
            # remaining tiles tile-major: evictions + output DMAs pipeline
            for t in range(GS, NT):
                pt = mktile(t)
                for m in range(6):
                    conv_mm(t, pt, m)
                evict(t, pt)

    nc.compile()
    return nc


def _get_nc():
    if "nc" not in _NC_CACHE:
        _NC_CACHE["nc"] = _build_nc()
    return _NC_CACHE["nc"]


def _prep_inputs(x, weight, conv_w, conv_b, net0_w, net0_b, net1_w, net1_b,
                 net2_w, net2_b):
    import ml_dtypes
    cw2 = _make_cw2(np.asarray(net0_w, np.float32), np.asarray(net0_b, np.float32),
                    np.asarray(net1_w, np.float32), np.asarray(net1_b, np.float32),
                    np.asarray(net2_w, np.float32), np.asarray(net2_b, np.float32))
    wf = np.asarray(weight, np.float32)
    cwf = np.asarray(conv_w, np.float32)
    # fold the static conv bank into every mixed bank (sum(att) == 1)
    banks = np.stack([_make_bank(wf[k] + cwf) for k in range(K)])  # (K,128,6,64)
    banks = np.ascontiguousarray(
        banks.transpose(1, 2, 0, 3)).astype(ml_dtypes.bfloat16)    # (128,6,K,64)
    convb = np.ascontiguousarray(np.asarray(conv_b, np.float32).reshape(C, 1))
    x = np.asarray(x, np.float32)
    xp = np.zeros((N, C, H, WP), np.float32)
    xp[:, :, :, :W] = x
    xs = xp.astype(ml_dtypes.bfloat16)
    in_maps = []
    for n in range(N):
        in_maps.append({
            "xin": np.ascontiguousarray(xs[n].reshape(C, H * WP)),
            "wbanks": banks,
            "cw2": cw2,
            "convb": convb,
        })
    return in_maps


def _run(inputs, trace=False, **kw):
    from concourse.bass_utils import run_bass_kernel_spmd
    nc = _get_nc()
    in_maps = _prep_inputs(**inputs)
    return run_bass_kernel_spmd(nc, in_maps, core_ids=list(range(N)), trace=trace, **kw)


def kernel(**inputs):
    res = _run(inputs)
    out = np.stack([res.results[n]["out"] for n in range(N)]).astype(np.float32)
    return out


# revision 18
# speedup vs baseline: 1.0615x; 1.0021x over previous
"""CondConv2d on 8 Trainium2 NeuronCores — data-parallel over batch N=8.

Per-core (one sample), all conv data in bf16:
  - x is read from HBM ONCE (lower partitions 0-63) in 8 chunks; the row-
    shifted upper copy (partitions 64-127) is produced by SBUF->SBUF DMAs
    whose descriptors drain behind the remaining loads — the conv consumes
    upper chunks far later than they arrive.
  - The attention branch (three global-mean-pooled conv3ds) collapses to a
    linear function of basis sums of x.  The expensive per-channel totals
    exploit that the conv3d depth masks only differentiate channels
    {0,1,62,63}: a selector matmul accumulates [4 edge-channel totals +
    grand total] into one PSUM bank on the otherwise-idle PE as chunks land,
    and one 512-wide DVE reduce + per-partition coefficients absorb them.
    Edge rows/cols/corners are tiny chunk-gated DVE ops writing columns of
    one [64,14] matrix; a fused 4-op DVE contraction produces the logits.
  - Softmax normalization is skipped: weights are mixed with raw exp(logits)
    (the static conv_w bank is pre-folded into each bank host-side since
    sum(att)=1), and the 1/sum(exp) scale is applied at PSUM eviction
    together with the conv bias.
  - The 3x3 conv runs as 43 PSUM tiles x 6 accumulating PE matmuls over a
    130-wide zero-padded layout; contraction 128 = 64 channels + 64 channels
    of the row-shifted copy, pairing taps (-1,w)+(0,w) per matmul.  The first
    5 tiles run m-outer (start needs only mixed bank 0); the rest run
    tile-major so evictions and output DMAs pipeline.
  - att-gated filler matmuls bridge the softmax/mixing window so the PE
    p-state stays ramped into the conv.
"""
import numpy as np

CONV_DT = "bf16"
N, C, H, W = 8, 64, 128, 128
K = 4
WP = W + 2                 # padded row width (130)
NELEM = WP * H + 262       # per-partition x buffer length (16902)
ROWS_PER_TILE = 3
GS = 5                     # conv tiles in the m-outer head group
NT = 43

NCHUNK = 8
CROWS = 16                 # rows per chunk

MM_TAPS = [((-1, -1), (0, -1)), ((-1, 0), (0, 0)), ((-1, 1), (0, 1)),
           ((1, -1), None), ((1, 0), None), ((1, 1), None)]
MM_OFFS = [130 * L[0] + L[1] for L, _ in MM_TAPS]

NBASIS = 14
SMM_W = 512                # selector-matmul free width
NSMM = (H * WP + SMM_W - 1) // SMM_W   # 33


# ----------------------------------------------------------------------------
# host-side prep
# ----------------------------------------------------------------------------
def _make_cw(net0_w, net0_b, net1_w, net1_b, net2_w, net2_b):
    """CW[c, b, k] over the 10 logical bases:
    0=total, 1=row0, 2=row127, 3=col0, 4=col127,
    5..8=corners (00,0W,H0,HW), 9=const 1."""
    cw = np.zeros((C, 10, K), np.float64)
    scale = 1.0 / (C * H * W)
    for w_net, pads in ((net0_w, (0, 0, 0)), (net1_w, (1, 1, 1)), (net2_w, (2, 1, 1))):
        Kk, _, kd, kh, kw = w_net.shape
        pd, ph, pw = pads
        for i in range(kd):
            clo, chi = max(0, i - pd), min(C - 1, C - 1 + i - pd)
            cmask = np.zeros(C)
            cmask[clo:chi + 1] = 1.0
            for j in range(kh):
                hlo, hhi = max(0, j - ph), min(H - 1, H - 1 + j - ph)
                dropA = 0 if hlo == 1 else (127 if hhi == H - 2 else None)
                for l in range(kw):
                    wlo, whi = max(0, l - pw), min(W - 1, W - 1 + l - pw)
                    dropB = 0 if wlo == 1 else (127 if whi == W - 2 else None)
                    v = np.zeros(10)
                    v[0] = 1.0
                    if dropA == 0: v[1] = -1.0
                    if dropA == 127: v[2] = -1.0
                    if dropB == 0: v[3] = -1.0
                    if dropB == 127: v[4] = -1.0
                    if dropA is not None and dropB is not None:
                        v[{(0, 0): 5, (0, 127): 6, (127, 0): 7, (127, 127): 8}[(dropA, dropB)]] = 1.0
                    for k in range(Kk):
                        cw[:, :, k] += w_net[k, 0, i, j, l] * scale * np.outer(cmask, v)
    btot = (net0_b + net1_b + net2_b).astype(np.float64)
    cw[:, 9, :] += btot[None, :] / C
    return cw


EDGE_CH = [0, 1, 62, 63]


def _make_cw2(cw):
    """Expand CW (C,10,K) to the 14 device basis columns:
    0 = PE selector column (partitions 0-3 = edge-channel totals,
        partition 4 = grand total), 1=row0, 2=row127, 3..5=col0 parts,
    6..8=col127 parts, 9..12=corners, 13=const."""
    cwmid = cw[C // 2, 0, :]
    assert np.abs(cw[2:62, 0, :] - cwmid[None, :]).max() < 1e-12
    cwx = np.zeros((C, NBASIS, K), np.float64)
    for i, e in enumerate(EDGE_CH):
        cwx[i, 0, :] = cw[e, 0, :] - cwmid
    cwx[4, 0, :] = cwmid
    exp_map = [1, 2, 3, 3, 3, 4, 4, 4, 5, 6, 7, 8, 9]
    cwx[:, 1:, :] = cw[:, exp_map, :]
    return np.ascontiguousarray(cwx.astype(np.float32))


def _make_bank(Wt):
    """Wt (co, ci, 3, 3) -> (128, 6, 64): [p=ci(lo)/64+ci(hi), mm, co]."""
    bank = np.zeros((128, 6, 64), np.float32)
    for m, (L, Hh) in enumerate(MM_TAPS):
        bank[:64, m, :] = Wt[:, :, 1 + L[0], 1 + L[1]].T
        if Hh is not None:
            bank[64:, m, :] = Wt[:, :, 1 + Hh[0], 1 + Hh[1]].T
    return bank


# ----------------------------------------------------------------------------
# device program
# ----------------------------------------------------------------------------
_NC_CACHE = {}


def _build_nc():
    import concourse.bacc as bacc
    import concourse.tile as tile
    from concourse import mybir

    f32 = mybir.dt.float32
    DT = mybir.dt.bfloat16
    Alu = mybir.AluOpType
    Ax = mybir.AxisListType
    Act = mybir.ActivationFunctionType

    nc = bacc.Bacc("TRN2", target_bir_lowering=False, debug=False,
                   enable_asserts=False, num_devices=N)
    xin = nc.dram_tensor("xin", [C, H * WP], DT, kind="ExternalInput")
    seld = nc.dram_tensor("sel", [C, 128], DT, kind="ExternalInput")
    wbk = nc.dram_tensor("wbanks", [128, 6, K, 64], DT, kind="ExternalInput")
    cw2 = nc.dram_tensor("cw2", [C, NBASIS, K], f32, kind="ExternalInput")
    cb = nc.dram_tensor("convb", [C, 1], f32, kind="ExternalInput")
    outT = nc.dram_tensor("out", [C, H, W], f32, kind="ExternalOutput")

    with tile.TileContext(nc) as tc:
        with tc.tile_pool(name="singles", bufs=1) as S, \
             tc.tile_pool(name="stage", bufs=4) as STG, \
             tc.tile_pool(name="spsum", bufs=1, space="PSUM") as PS1, \
             tc.tile_pool(name="cpsum", bufs=GS, space="PSUM") as PS:

            XL = S.tile([128, NELEM], DT)
            wb_sb = S.tile([128, 6, K, 64], DT)
            cw2_sb = S.tile([C, NBASIS, K], f32)
            convb_sb = S.tile([C, 1], f32)
            zlhs = S.tile([128, 128], DT)
            sel = S.tile([64, 128], DT)
            onesall = S.tile([C, 128], f32)
            att_sb = S.tile([128, K], f32)
            M = S.tile([C, NBASIS], f32)
            G = S.tile([C, K], f32)
            gscr = S.tile([C, NBASIS], f32)
            mw = S.tile([128, 6, 64], f32)
            mwb = S.tile([128, 6, 64], DT)
            actscr = S.tile([C, 136], f32)
            attw = S.tile([64, 390], DT)
            ssum = S.tile([128, 1], f32)
            sinv = S.tile([128, 1], f32)

            psum_s = PS1.tile([128, SMM_W], f32)
            wpsum = PS1.tile([128, 512], f32)
            psum_b = PS1.tile([128, K], f32)

            # --- constants / border zeroing (all tiny) ---
            nc.vector.memset(zlhs, 0.0)
            nc.vector.memset(onesall, 1.0)
            nc.vector.memset(M[:, 13:14], 1.0)
            # borders: host pre-pads the row gaps; only head/tail need zeroing
            nc.vector.memset(XL[0:64, 0:132], 0.0)
            nc.vector.memset(XL[0:64, 132 + H * WP:NELEM], 0.0)
            nc.vector.memset(XL[64:128, 0:2], 0.0)
            nc.vector.memset(XL[64:128, 2 + H * WP:NELEM], 0.0)

            # --- PE pipeline warm-up (results discarded; zlhs is all-zero) ---
            for i in range(8):
                nc.tensor.matmul(wpsum[:, 0:128], zlhs, zlhs, start=True, stop=True)

            # --- input DMAs, all on one ring so queue order is exact:
            # x lower chunks first, then coefficients/banks, then the
            # SBUF->SBUF row-shifted upper copies (conv needs them late).
            nc.sync.dma_start(out=sel, in_=seld[:, :])
            for c in range(NCHUNK):
                a = WP * CROWS * c
                ln = WP * CROWS
                nc.sync.dma_start(out=XL[0:64, 132 + a: 132 + a + ln],
                                  in_=xin[:, a: a + ln])
            nc.sync.dma_start(out=cw2_sb, in_=cw2[:, :, :])
            nc.sync.dma_start(out=convb_sb, in_=cb[:, :])
            for m in range(6):
                nc.sync.dma_start(out=wb_sb[:, m, :, :], in_=wbk[:, m, :, :])
            for c in range(NCHUNK):
                a = WP * CROWS * c
                ln = WP * CROWS
                nc.sync.dma_start(out=XL[64:128, 2 + a: 2 + a + ln],
                                  in_=XL[0:64, 132 + a: 132 + a + ln])

            # --- per-channel totals on the PE: accumulate selector matmuls
            # over x as chunks land; psum_s row i = total of edge channel i,
            # row 4 = grand total.
            for j in range(NSMM):
                a = 132 + SMM_W * j
                fw = min(SMM_W, 132 + H * WP - a)
                nc.tensor.matmul(psum_s[:, :fw], sel, XL[0:64, a:a + fw],
                                 start=(j == 0), stop=(j == NSMM - 1))

            # --- small basis sums (chunk-gated) ---
            def colpart(col, r0, r1, mcol):
                a = 132 + WP * r0 + col
                v = XL[0:64, a:a + WP * (r1 - r0)].rearrange(
                    "p (r w) -> p r w", w=WP)[:, :, 0:1]
                nc.vector.tensor_reduce(out=M[:, mcol:mcol + 1], in_=v,
                                        axis=Ax.XY, op=Alu.add)

            # after chunk 0: row0 sum + row-0 corners
            nc.vector.tensor_reduce(out=M[:, 1:2], in_=XL[0:64, 132:132 + W],
                                    axis=Ax.X, op=Alu.add)
            nc.vector.tensor_copy(
                out=M[:, 9:11].rearrange("p (a b) -> p a b", b=1),
                in_=XL[0:64, 132:132 + 254].rearrange("p (a b) -> p a b", b=127)[:, :, 0:1])
            # col parts: rows [0,64) after chunk 3, [64,112) after chunk 6,
            # [112,128) after chunk 7
            colpart(0, 0, 64, 3)
            colpart(127, 0, 64, 6)
            colpart(0, 64, 112, 4)
            colpart(127, 64, 112, 7)
            colpart(0, 112, 128, 5)
            colpart(127, 112, 128, 8)
            # last chunk: row127 + corners on the idle ACT engine
            nc.scalar.activation(out=actscr[:, 0:W], in_=XL[0:64, 16642:16642 + W],
                                 func=Act.Identity, accum_out=M[:, 2:3])
            nc.scalar.copy(
                out=M[:, 11:13].rearrange("p (a b) -> p a b", b=1),
                in_=XL[0:64, 16642:16642 + 254].rearrange("p (a b) -> p a b", b=127)[:, :, 0:1])
            # PE totals column: reduce the accumulated selector PSUM
            nc.vector.tensor_reduce(out=M[:, 0:1], in_=psum_s[0:64, :],
                                    axis=Ax.X, op=Alu.add)

            # per-channel coefficient contraction: G[c,k] = sum_b M[c,b]*CW2[c,b,k]
            for k in range(K):
                nc.vector.scalar_tensor_tensor(
                    out=gscr, in0=M, scalar=1.0,
                    in1=cw2_sb[:, :, k], op0=Alu.mult, op1=Alu.mult,
                    accum_out=G[:, k:k + 1])

            # logits broadcast to all 128 partitions with one matmul
            nc.tensor.matmul(psum_b, onesall, G, start=True, stop=True)
            # unnormalized softmax: att = exp(logits); 1/sum applied at eviction
            nc.scalar.activation(out=att_sb, in_=psum_b, func=Act.Exp)
            # PE keep-warm bridge across softmax+mixing: fillers whose deps
            # only clear late (att_sb after EXP, then a wide ACT-made scratch)
            att_bc = att_sb.bitcast(DT)
            for i in range(3):
                nc.tensor.matmul(wpsum[:, 0:2 * K], zlhs, att_bc, start=True, stop=True)
            nc.scalar.activation(out=attw, in_=XL[0:64, 132:132 + 390],
                                 func=Act.Identity, scale=att_sb[0:64, 0:1])
            for i in range(4):
                nc.tensor.matmul(wpsum[:, 0:390], zlhs[0:64, :], attw,
                                 start=True, stop=True)

            # --- weight mixing: mwb[:,m,:] = sum_k exp_k * bank'_k[:,m,:] ---
            def mixbank(m):
                nc.vector.tensor_scalar_mul(out=mw[:, m, :], in0=wb_sb[:, m, 0, :],
                                            scalar1=att_sb[:, 0:1])
                for k in range(1, K):
                    tgt = mwb if k == K - 1 else mw
                    nc.vector.scalar_tensor_tensor(
                        out=tgt[:, m, :], in0=wb_sb[:, m, k, :],
                        scalar=att_sb[:, k:k + 1], in1=mw[:, m, :],
                        op0=Alu.mult, op1=Alu.add)

            mixbank(0)
            nc.vector.tensor_reduce(out=ssum, in_=att_sb, axis=Ax.X, op=Alu.add)
            nc.vector.reciprocal(out=sinv, in_=ssum)
            for m in range(1, 6):
                mixbank(m)

            # --- main conv ---
            def evict(t, pt):
                r0 = 1 + t * ROWS_PER_TILE
                nrows = min(ROWS_PER_TILE, H + 1 - r0)
                F = WP * nrows
                st = STG.tile([64, WP * ROWS_PER_TILE], f32,
                              tag="stg", name=f"stg{t}")
                if t % 2 == 0:
                    nc.scalar.activation(out=st[:, :F], in_=pt,
                                         func=Act.Identity,
                                         bias=convb_sb[:, 0:1],
                                         scale=sinv[0:64, 0:1])
                else:
                    nc.vector.tensor_scalar(
                        out=st[:, :F], in0=pt,
                        scalar1=sinv[0:64, 0:1],
                        scalar2=convb_sb[:, 0:1],
                        op0=Alu.mult, op1=Alu.add)
                src = st[:, :F].rearrange("p (r w) -> p r w", w=WP)[:, :, 1:1 + W]
                eng = nc.sync if t % 2 == 0 else nc.scalar
                eng.dma_start(out=outT[:, r0 - 1:r0 - 1 + nrows, :], in_=src)

            def conv_mm(t, pt, m):
                r0 = 1 + t * ROWS_PER_TILE
                F = WP * min(ROWS_PER_TILE, H + 1 - r0)
                rhs = XL[:, WP * r0 + MM_OFFS[m] + 1:
                         WP * r0 + MM_OFFS[m] + 1 + F]
                nc.tensor.matmul(pt, mwb[:, m, :], rhs,
                                 start=(m == 0), stop=(m == 5))

            def mktile(t):
                F = WP * min(ROWS_PER_TILE, H + 1 - (1 + t * ROWS_PER_TILE))
                return PS.tile([64, WP * ROWS_PER_TILE], f32,
                               tag="cps", name=f"cps{t}")[:, :F]

            # head group m-outer: the first matmuls only need mixed bank 0
            pts = {t: mktile(t) for t in range(GS)}
            for m in range(6):
                for t in range(GS):
                    conv_mm(t, pts[t], m)
            for t in range(GS):
                evict(t, pts[t])
            # remaining tiles tile-major: evictions + output DMAs pipeline
            for t in range(GS, NT):
                pt = mktile(t)
                for m in range(6):
                    conv_mm(t, pt, m)
                evict(t, pt)

    nc.compile()
    return nc


def _get_nc():
    if "nc" not in _NC_CACHE:
        _NC_CACHE["nc"] = _build_nc()
    return _NC_CACHE["nc"]


def _prep_inputs(x, weight, conv_w, conv_b, net0_w, net0_b, net1_w, net1_b,
                 net2_w, net2_b):
    import ml_dtypes
    cw = _make_cw(np.asarray(net0_w, np.float32), np.asarray(net0_b, np.float32),
                  np.asarray(net1_w, np.float32), np.asarray(net1_b, np.float32),
                  np.asarray(net2_w, np.float32), np.asarray(net2_b, np.float32))
    cw2 = _make_cw2(cw)
    wf = np.asarray(weight, np.float32)
    cwf = np.asarray(conv_w, np.float32)
    # fold the static conv bank into every mixed bank (sum(att) == 1)
    banks = np.stack([_make_bank(wf[k] + cwf) for k in range(K)])  # (K,128,6,64)
    banks = np.ascontiguousarray(
        banks.transpose(1, 2, 0, 3)).astype(ml_dtypes.bfloat16)    # (128,6,K,64)
    convb = np.ascontiguousarray(np.asarray(conv_b, np.float32).reshape(C, 1))
    selh = np.zeros((C, 128), np.float32)
    for i, e in enumerate(EDGE_CH):
        selh[e, i] = 1.0
    selh[:, 4] = 1.0
    selh = np.ascontiguousarray(selh.astype(ml_dtypes.bfloat16))
    x = np.asarray(x, np.float32)
    xp = np.zeros((N, C, H, WP), np.float32)
    xp[:, :, :, :W] = x
    xs = xp.astype(ml_dtypes.bfloat16)
    in_maps = []
    for n in range(N):
        in_maps.append({
            "xin": np.ascontiguousarray(xs[n].reshape(C, H * WP)),
            "sel": selh,
            "wbanks": banks,
            "cw2": cw2,
            "convb": convb,
        })
    return in_maps


def _run(inputs, trace=False, **kw):
    from concourse.bass_utils import run_bass_kernel_spmd
    nc = _get_nc()
    in_maps = _prep_inputs(**inputs)
    return run_bass_kernel_spmd(nc, in_maps, core_ids=list(range(N)), trace=trace, **kw)


def kernel(**inputs):
    res = _run(inputs)
    out = np.stack([res.results[n]["out"] for n in range(N)]).astype(np.float32)
    return out


# revision 26
# speedup vs baseline: 1.1796x; 1.1113x over previous
"""CondConv2d on 8 Trainium2 NeuronCores — data-parallel over batch N=8.

Per-core (one sample), all conv data in bf16:
  - x is read from HBM ONCE (lower partitions 0-63) in 8 chunks; the row-
    shifted upper copy (partitions 64-127) is produced by SBUF->SBUF DMAs
    whose descriptors drain behind the remaining loads — the conv consumes
    upper chunks far later than they arrive.
  - The attention branch (three global-mean-pooled conv3ds) collapses to a
    linear function of basis sums of x.  The expensive per-channel totals
    exploit that the conv3d depth masks only differentiate channels
    {0,1,62,63}: a selector matmul accumulates [4 edge-channel totals +
    grand total] into one PSUM bank on the otherwise-idle PE as chunks land,
    and one 512-wide DVE reduce + per-partition coefficients absorb them.
    Edge rows/cols/corners are tiny chunk-gated DVE ops writing columns of
    one [64,14] matrix; a fused 4-op DVE contraction produces the logits.
  - Softmax normalization is skipped: weights are mixed with raw exp(logits)
    (the static conv_w bank is pre-folded into each bank host-side since
    sum(att)=1), and the 1/sum(exp) scale is applied at PSUM eviction
    together with the conv bias.
  - The 3x3 conv runs as 43 PSUM tiles x 6 accumulating PE matmuls over a
    130-wide zero-padded layout; contraction 128 = 64 channels + 64 channels
    of the row-shifted copy, pairing taps (-1,w)+(0,w) per matmul.  The first
    5 tiles run m-outer (start needs only mixed bank 0); the rest run
    tile-major so evictions and output DMAs pipeline.
  - att-gated filler matmuls bridge the softmax/mixing window so the PE
    p-state stays ramped into the conv.
"""
import numpy as np

CONV_DT = "bf16"
N, C, H, W = 8, 64, 128, 128
K = 4
WP = W + 2                 # padded row width (130)
NELEM = WP * H + 262       # per-partition x buffer length (16902)
ROWS_PER_TILE = 3
GS = 5                     # conv tiles in the m-outer head group
NT = 43

NCHUNK = 8
CROWS = 16                 # rows per chunk

MM_TAPS = [((-1, -1), (0, -1)), ((-1, 0), (0, 0)), ((-1, 1), (0, 1)),
           ((1, -1), None), ((1, 0), None), ((1, 1), None)]
MM_OFFS = [130 * L[0] + L[1] for L, _ in MM_TAPS]

NBASIS = 18
SMM_W = 512                # selector-matmul free width
NSMM = 16                  # selector matmuls cover elems [0, 8192) (rows 0-63)
SMM_END = SMM_W * NSMM
# DVE fold spans for the rest of the totals (span, gating is automatic)
FOLD_SPANS = [(8192, 10400), (10400, 12480), (12480, 14560), (14560, 16640)]


# ----------------------------------------------------------------------------
# host-side prep
# ----------------------------------------------------------------------------
def _make_cw(net0_w, net0_b, net1_w, net1_b, net2_w, net2_b):
    """CW[c, b, k] over the 10 logical bases:
    0=total, 1=row0, 2=row127, 3=col0, 4=col127,
    5..8=corners (00,0W,H0,HW), 9=const 1."""
    cw = np.zeros((C, 10, K), np.float64)
    scale = 1.0 / (C * H * W)
    for w_net, pads in ((net0_w, (0, 0, 0)), (net1_w, (1, 1, 1)), (net2_w, (2, 1, 1))):
        Kk, _, kd, kh, kw = w_net.shape
        pd, ph, pw = pads
        for i in range(kd):
            clo, chi = max(0, i - pd), min(C - 1, C - 1 + i - pd)
            cmask = np.zeros(C)
            cmask[clo:chi + 1] = 1.0
            for j in range(kh):
                hlo, hhi = max(0, j - ph), min(H - 1, H - 1 + j - ph)
                dropA = 0 if hlo == 1 else (127 if hhi == H - 2 else None)
                for l in range(kw):
                    wlo, whi = max(0, l - pw), min(W - 1, W - 1 + l - pw)
                    dropB = 0 if wlo == 1 else (127 if whi == W - 2 else None)
                    v = np.zeros(10)
                    v[0] = 1.0
                    if dropA == 0: v[1] = -1.0
                    if dropA == 127: v[2] = -1.0
                    if dropB == 0: v[3] = -1.0
                    if dropB == 127: v[4] = -1.0
                    if dropA is not None and dropB is not None:
                        v[{(0, 0): 5, (0, 127): 6, (127, 0): 7, (127, 127): 8}[(dropA, dropB)]] = 1.0
                    for k in range(Kk):
                        cw[:, :, k] += w_net[k, 0, i, j, l] * scale * np.outer(cmask, v)
    btot = (net0_b + net1_b + net2_b).astype(np.float64)
    cw[:, 9, :] += btot[None, :] / C
    return cw


EDGE_CH = [0, 1, 62, 63]


def _make_cw2(cw):
    """Expand CW (C,10,K) to the 18 device basis columns:
    0 = PE selector column over elems [0,SMM_END) (partitions 0-3 =
        edge-channel partials, partition 4 = mid-channel grand partial),
    1=row0, 2=row127, 3..5=col0 parts, 6..8=col127 parts, 9..12=corners,
    13=const, 14..17 = DVE per-channel total folds over FOLD_SPANS."""
    cwmid = cw[C // 2, 0, :]
    assert np.abs(cw[2:62, 0, :] - cwmid[None, :]).max() < 1e-12
    cwx = np.zeros((C, NBASIS, K), np.float64)
    for i, e in enumerate(EDGE_CH):
        cwx[i, 0, :] = cw[e, 0, :] - cwmid
    cwx[4, 0, :] = cwmid
    exp_map = [1, 2, 3, 3, 3, 4, 4, 4, 5, 6, 7, 8, 9]
    cwx[:, 1:14, :] = cw[:, exp_map, :]
    cwx[:, 14:18, :] = cw[:, [0, 0, 0, 0], :]
    return np.ascontiguousarray(cwx.astype(np.float32))


def _make_bank(Wt):
    """Wt (co, ci, 3, 3) -> (128, 6, 64): [p=ci(lo)/64+ci(hi), mm, co]."""
    bank = np.zeros((128, 6, 64), np.float32)
    for m, (L, Hh) in enumerate(MM_TAPS):
        bank[:64, m, :] = Wt[:, :, 1 + L[0], 1 + L[1]].T
        if Hh is not None:
            bank[64:, m, :] = Wt[:, :, 1 + Hh[0], 1 + Hh[1]].T
    return bank


# ----------------------------------------------------------------------------
# device program
# ----------------------------------------------------------------------------
_NC_CACHE = {}


def _build_nc():
    import concourse.bacc as bacc
    import concourse.tile as tile
    from concourse import mybir

    f32 = mybir.dt.float32
    DT = mybir.dt.bfloat16
    Alu = mybir.AluOpType
    Ax = mybir.AxisListType
    Act = mybir.ActivationFunctionType

    nc = bacc.Bacc("TRN2", target_bir_lowering=False, debug=False,
                   enable_asserts=False, num_devices=N)
    xin = nc.dram_tensor("xin", [C, H * WP], DT, kind="ExternalInput")
    seld = nc.dram_tensor("sel", [C, 128], DT, kind="ExternalInput")
    wbk = nc.dram_tensor("wbanks", [128, 6, K, 64], DT, kind="ExternalInput")
    cw2 = nc.dram_tensor("cw2", [C, NBASIS, K], f32, kind="ExternalInput")
    cb = nc.dram_tensor("convb", [C, 1], f32, kind="ExternalInput")
    outT = nc.dram_tensor("out", [C, H, W], f32, kind="ExternalOutput")

    with tile.TileContext(nc) as tc:
        with tc.tile_pool(name="singles", bufs=1) as S, \
             tc.tile_pool(name="stage", bufs=6) as STG, \
             tc.tile_pool(name="spsum", bufs=1, space="PSUM") as PS1, \
             tc.tile_pool(name="cpsum", bufs=GS, space="PSUM") as PS:

            XL = S.tile([128, NELEM], DT)
            wb_sb = S.tile([128, 6, K, 64], DT)
            cw2_sb = S.tile([C, NBASIS, K], f32)
            convb_sb = S.tile([C, 1], f32)
            zlhs = S.tile([128, 128], DT)
            sel = S.tile([64, 128], DT)
            onesall = S.tile([C, 128], f32)
            att_sb = S.tile([128, K], f32)
            M = S.tile([C, NBASIS], f32)
            G = S.tile([C, K], f32)
            gscr = S.tile([C, NBASIS], f32)
            mw = S.tile([128, 6, 64], f32)
            mwb = S.tile([128, 6, 64], DT)
            actscr = S.tile([C, 528], f32)
            foldA = S.tile([C, 1104], DT)
            foldB = S.tile([C, 1104], DT)
            attw = S.tile([64, 390], DT)
            ssum = S.tile([128, 1], f32)
            sinv = S.tile([128, 1], f32)

            psum_s = PS1.tile([128, SMM_W], f32)
            wpsum = PS1.tile([128, 512], f32)
            psum_b = PS1.tile([128, K], f32)

            # --- constants / border zeroing (all tiny) ---
            nc.vector.memset(zlhs, 0.0)
            nc.vector.memset(onesall, 1.0)
            nc.vector.memset(M[:, 13:14], 1.0)
            # borders: host pre-pads the row gaps; only head/tail need zeroing
            nc.vector.memset(XL[0:64, 0:132], 0.0)
            nc.vector.memset(XL[0:64, 132 + H * WP:NELEM], 0.0)
            nc.vector.memset(XL[64:128, 0:2], 0.0)
            nc.vector.memset(XL[64:128, 2 + H * WP:NELEM], 0.0)

            # --- PE pipeline warm-up (results discarded; zlhs is all-zero) ---
            for i in range(8):
                nc.tensor.matmul(wpsum[:, 0:128], zlhs, zlhs, start=True, stop=True)

            # --- input DMAs, all on one ring so queue order is exact:
            # x lower chunks first, then coefficients/banks, then the
            # SBUF->SBUF row-shifted upper copies (conv needs them late).
            nc.sync.dma_start(out=sel, in_=seld[:, :])
            for c in range(NCHUNK):
                a = WP * CROWS * c
                ln = WP * CROWS
                nc.sync.dma_start(out=XL[0:64, 132 + a: 132 + a + ln],
                                  in_=xin[:, a: a + ln])
            nc.sync.dma_start(out=cw2_sb, in_=cw2[:, :, :])
            nc.sync.dma_start(out=convb_sb, in_=cb[:, :])
            for m in range(6):
                nc.sync.dma_start(out=wb_sb[:, m, :, :], in_=wbk[:, m, :, :])
            for c in range(NCHUNK):
                a = WP * CROWS * c
                ln = WP * CROWS
                nc.sync.dma_start(out=XL[64:128, 2 + a: 2 + a + ln],
                                  in_=XL[0:64, 132 + a: 132 + a + ln])

            # --- per-channel totals, split PE/DVE: the PE accumulates
            # selector matmuls over elems [0, SMM_END) as chunks land
            # (psum_s row i = edge-channel partial, row 4 = grand partial);
            # the DVE folds the remaining spans per-channel.
            for j in range(NSMM):
                a = 132 + SMM_W * j
                nc.tensor.matmul(psum_s, sel, XL[0:64, a:a + SMM_W],
                                 start=(j == 0), stop=(j == NSMM - 1))

            # --- small basis sums (chunk-gated) ---
            def colpart(col, r0, r1, mcol):
                a = 132 + WP * r0 + col
                v = XL[0:64, a:a + WP * (r1 - r0)].rearrange(
                    "p (r w) -> p r w", w=WP)[:, :, 0:1]
                nc.vector.tensor_reduce(out=M[:, mcol:mcol + 1], in_=v,
                                        axis=Ax.XY, op=Alu.add)

            def fold(i, obuf):
                a, b = FOLD_SPANS[i]
                h = (b - a) // 2
                nc.vector.scalar_tensor_tensor(
                    out=obuf[:, :h], in0=XL[0:64, 132 + a:132 + a + h], scalar=1.0,
                    in1=XL[0:64, 132 + a + h:132 + b], op0=Alu.mult, op1=Alu.add,
                    accum_out=M[:, 14 + i:15 + i])

            # after chunk 0: row0 sum + row-0 corners
            nc.vector.tensor_reduce(out=M[:, 1:2], in_=XL[0:64, 132:132 + W],
                                    axis=Ax.X, op=Alu.add)
            nc.vector.tensor_copy(
                out=M[:, 9:11].rearrange("p (a b) -> p a b", b=1),
                in_=XL[0:64, 132:132 + 254].rearrange("p (a b) -> p a b", b=127)[:, :, 0:1])
            # col parts rows [0,64); then totals folds + col parts, paced so
            # only ~0.6us of DVE work remains after the last chunk lands
            colpart(0, 0, 64, 3)
            colpart(127, 0, 64, 6)
            fold(0, foldA)
            fold(1, foldB)
            colpart(0, 64, 112, 4)
            colpart(127, 64, 112, 7)
            fold(2, foldA)
            fold(3, foldB)
            # ACT (idle): selector-PSUM reduce, then last-chunk smalls
            nc.scalar.activation(out=actscr[:, 0:SMM_W], in_=psum_s[0:64, :],
                                 func=Act.Identity, accum_out=M[:, 0:1])
            for col, mcol in ((0, 5), (127, 8)):
                a = 132 + WP * 112 + col
                v = XL[0:64, a:a + WP * 16].rearrange("p (r w) -> p r w", w=WP)[:, :, 0:1]
                nc.scalar.activation(out=actscr[:, 512:528].rearrange(
                    "p (r w) -> p r w", w=1), in_=v,
                    func=Act.Identity, accum_out=M[:, mcol:mcol + 1])
            nc.scalar.activation(out=actscr[:, 0:W], in_=XL[0:64, 16642:16642 + W],
                                 func=Act.Identity, accum_out=M[:, 2:3])
            nc.scalar.copy(
                out=M[:, 11:13].rearrange("p (a b) -> p a b", b=1),
                in_=XL[0:64, 16642:16642 + 254].rearrange("p (a b) -> p a b", b=127)[:, :, 0:1])

            # per-channel coefficient contraction: G[c,k] = sum_b M[c,b]*CW2[c,b,k]
            for k in range(K):
                nc.vector.scalar_tensor_tensor(
                    out=gscr, in0=M, scalar=1.0,
                    in1=cw2_sb[:, :, k], op0=Alu.mult, op1=Alu.mult,
                    accum_out=G[:, k:k + 1])

            # logits broadcast to all 128 partitions with one matmul
            nc.tensor.matmul(psum_b, onesall, G, start=True, stop=True)
            # unnormalized softmax: att = exp(logits); 1/sum applied at eviction
            nc.scalar.activation(out=att_sb, in_=psum_b, func=Act.Exp)
            # PE keep-warm bridge across softmax+mixing: fillers whose deps
            # only clear late (att_sb after EXP, then a wide ACT-made scratch)
            att_bc = att_sb.bitcast(DT)
            for i in range(3):
                nc.tensor.matmul(wpsum[:, 0:2 * K], zlhs, att_bc, start=True, stop=True)
            nc.scalar.activation(out=attw, in_=XL[0:64, 132:132 + 390],
                                 func=Act.Identity, scale=att_sb[0:64, 0:1])
            for i in range(4):
                nc.tensor.matmul(wpsum[:, 0:390], zlhs[0:64, :], attw,
                                 start=True, stop=True)

            # --- weight mixing: mwb[:,m,:] = sum_k exp_k * bank'_k[:,m,:] ---
            def mixbank(m):
                nc.vector.tensor_scalar_mul(out=mw[:, m, :], in0=wb_sb[:, m, 0, :],
                                            scalar1=att_sb[:, 0:1])
                for k in range(1, K):
                    tgt = mwb if k == K - 1 else mw
                    nc.vector.scalar_tensor_tensor(
                        out=tgt[:, m, :], in0=wb_sb[:, m, k, :],
                        scalar=att_sb[:, k:k + 1], in1=mw[:, m, :],
                        op0=Alu.mult, op1=Alu.add)

            mixbank(0)
            nc.vector.tensor_reduce(out=ssum, in_=att_sb, axis=Ax.X, op=Alu.add)
            nc.vector.reciprocal(out=sinv, in_=ssum)
            for m in range(1, 6):
                mixbank(m)

            # --- main conv ---
            def evict(t, pt):
                r0 = 1 + t * ROWS_PER_TILE
                nrows = min(ROWS_PER_TILE, H + 1 - r0)
                F = WP * nrows
                st = STG.tile([64, WP * ROWS_PER_TILE], f32,
                              tag="stg", name=f"stg{t}")
                if t % 2 == 0:
                    nc.scalar.activation(out=st[:, :F], in_=pt,
                                         func=Act.Identity,
                                         bias=convb_sb[:, 0:1],
                                         scale=sinv[0:64, 0:1])
                else:
                    nc.vector.tensor_scalar(
                        out=st[:, :F], in0=pt,
                        scalar1=sinv[0:64, 0:1],
                        scalar2=convb_sb[:, 0:1],
                        op0=Alu.mult, op1=Alu.add)
                src = st[:, :F].rearrange("p (r w) -> p r w", w=WP)[:, :, 1:1 + W]
                eng = nc.sync if t % 2 == 0 else nc.scalar
                eng.dma_start(out=outT[:, r0 - 1:r0 - 1 + nrows, :], in_=src)

            def conv_mm(t, pt, m):
                r0 = 1 + t * ROWS_PER_TILE
                F = WP * min(ROWS_PER_TILE, H + 1 - r0)
                rhs = XL[:, WP * r0 + MM_OFFS[m] + 1:
                         WP * r0 + MM_OFFS[m] + 1 + F]
                nc.tensor.matmul(pt, mwb[:, m, :], rhs,
                                 start=(m == 0), stop=(m == 5))

            def mktile(t):
                F = WP * min(ROWS_PER_TILE, H + 1 - (1 + t * ROWS_PER_TILE))
                return PS.tile([64, WP * ROWS_PER_TILE], f32,
                               tag="cps", name=f"cps{t}")[:, :F]

            # head group m-outer: the first matmuls only need mixed bank 0
            pts = {t: mktile(t) for t in range(GS)}
            for m in range(6):
                for t in range(GS):
                    conv_mm(t, pts[t], m)
            for t in range(GS):
                evict(t, pts[t])
            # remaining tiles tile-major: evictions + output DMAs pipeline
            for t in range(GS, NT - 1):
                pt = mktile(t)
                for m in range(6):
                    conv_mm(t, pt, m)
                evict(t, pt)
            # last tile: split the eviction + output DMA across both engines
            # and both rings so the drain is as short as possible
            t = NT - 1
            pt = mktile(t)
            for m in range(6):
                conv_mm(t, pt, m)
            r0 = 1 + t * ROWS_PER_TILE
            st = STG.tile([64, WP * ROWS_PER_TILE], f32, tag="stg", name="stglast")
            nc.scalar.activation(out=st[:, 0:WP], in_=pt[:, 0:WP],
                                 func=Act.Identity, bias=convb_sb[:, 0:1],
                                 scale=sinv[0:64, 0:1])
            nc.vector.tensor_scalar(out=st[:, WP:2 * WP], in0=pt[:, WP:2 * WP],
                                    scalar1=sinv[0:64, 0:1],
                                    scalar2=convb_sb[:, 0:1],
                                    op0=Alu.mult, op1=Alu.add)
            nc.sync.dma_start(out=outT[:, r0 - 1:r0, :],
                              in_=st[:, 0:WP].rearrange("p (r w) -> p r w",
                                                        w=WP)[:, :, 1:1 + W])
            nc.scalar.dma_start(out=outT[:, r0:r0 + 1, :],
                                in_=st[:, WP:2 * WP].rearrange("p (r w) -> p r w",
                                                               w=WP)[:, :, 1:1 + W])

    nc.compile()
    return nc


def _get_nc():
    if "nc" not in _NC_CACHE:
        _NC_CACHE["nc"] = _build_nc()
    return _NC_CACHE["nc"]


def _prep_inputs(x, weight, conv_w, conv_b, net0_w, net0_b, net1_w, net1_b,
                 net2_w, net2_b):
    import ml_dtypes
    cw = _make_cw(np.asarray(net0_w, np.float32), np.asarray(net0_b, np.float32),
                  np.asarray(net1_w, np.float32), np.asarray(net1_b, np.float32),
                  np.asarray(net2_w, np.float32), np.asarray(net2_b, np.float32))
    cw2 = _make_cw2(cw)
    wf = np.asarray(weight, np.float32)
    cwf = np.asarray(conv_w, np.float32)
    # fold the static conv bank into every mixed bank (sum(att) == 1)
    banks = np.stack([_make_bank(wf[k] + cwf) for k in range(K)])  # (K,128,6,64)
    banks = np.ascontiguousarray(
        banks.transpose(1, 2, 0, 3)).astype(ml_dtypes.bfloat16)    # (128,6,K,64)
    convb = np.ascontiguousarray(np.asarray(conv_b, np.float32).reshape(C, 1))
    selh = np.zeros((C, 128), np.float32)
    for i, e in enumerate(EDGE_CH):
        selh[e, i] = 1.0
    selh[:, 4] = 1.0
    selh = np.ascontiguousarray(selh.astype(ml_dtypes.bfloat16))
    x = np.asarray(x, np.float32)
    xp = np.zeros((N, C, H, WP), np.float32)
    xp[:, :, :, :W] = x
    xs = xp.astype(ml_dtypes.bfloat16)
    in_maps = []
    for n in range(N):
        in_maps.append({
            "xin": np.ascontiguousarray(xs[n].reshape(C, H * WP)),
            "sel": selh,
            "wbanks": banks,
            "cw2": cw2,
            "convb": convb,
        })
    return in_maps


def _run(inputs, trace=False, **kw):
    from concourse.bass_utils import run_bass_kernel_spmd
    nc = _get_nc()
    in_maps = _prep_inputs(**inputs)
    return run_bass_kernel_spmd(nc, in_maps, core_ids=list(range(N)), trace=trace, **kw)


def kernel(**inputs):
    res = _run(inputs)
    out = np.stack([res.results[n]["out"] for n in range(N)]).astype(np.float32)
    return out
